# revision 1
# baseline (speedup 1.0000x reference)
"""HMLSTMOutput kernel for 8 TRN2 NeuronCores.

Data-parallel over tokens: core c handles 512 of the 4096 flattened tokens.
Per core, the whole pipeline runs feature-major ([feature, token] tiles):

  g = sigmoid(x @ w^T)                       [3, 512] gates
  x' = x * g (per 1024-feature block)        via PE-broadcast of g rows
  s = x'^T@emb_w + sum(emb_b); h = relu(s)   K=3072 GEMM
  h = tanh(h@lin_w[i] + lin_b[i])  (x2)      K=2048 GEMMs
  logits^T = out_w^T@h + out_b               K=2048, M=32000 GEMM (streamed W)

All matmuls in bf16 with fp32 PSUM accumulation. Weights are pre-chunked on
host into [128, K/128, M] partition-major layouts so every DMA line is
contiguous. Output is written vocab-major [250, 128, 512] per core and
re-assembled/transposed on host.
"""

import sys

sys.path.insert(0, "/opt/trn_rl_repo")

import numpy as np
import ml_dtypes

import concourse.bass as bass
import concourse.mybir as mybir
from concourse.tile import TileContext
from concourse.bass_utils import run_bass_kernel_spmd

F32 = mybir.dt.float32
BF16 = mybir.dt.bfloat16
AF = mybir.ActivationFunctionType

B, T, L, D_IN = 4, 1024, 3, 1024
D = L * D_IN            # 3072
EMB = 2048
OUT = 32000
NTOK = B * T            # 4096
NCORES = 8
TPC = NTOK // NCORES    # 512 tokens per core
KD = D // 128           # 24
KE = EMB // 128         # 16
VT = OUT // 128         # 250 vocab tiles


# ---------------------------------------------------------------- legalize
_lw_counter = [0]


def _mk_nop(engine, wait, base_name):
    _lw_counter[0] += 1
    return mybir.InstNoOp(
        name=f"{base_name}-lw{_lw_counter[0]}",
        engine=engine,
        ins=[],
        outs=[],
        sync_info=mybir.SyncInfo(on_wait=[wait], on_update=[]),
    )


def legalize_waits(nc, max_waits=1):
    """Split multi-wait instructions into single-wait NoOp chains (this
    walrus build allows ~1 wait + 1 update per instruction)."""
    for f in nc.m.functions:
        for bb in f.blocks:
            out = []
            changed = False
            for inst in bb.instructions:
                si = inst.sync_info
                if si is not None and si.on_wait and len(si.on_wait) > max_waits:
                    waits = list(si.on_wait)
                    keep_idx = len(waits) - 1
                    for i, w in enumerate(waits):
                        nm = getattr(w, "ant_name", None) or ""
                        if not ("DMAHW" in nm or "DMASW" in nm):
                            keep_idx = i
                            break
                    keep = waits[keep_idx]
                    rest = [w for i, w in enumerate(waits) if i != keep_idx]
                    for w in rest:
                        out.append(_mk_nop(inst.engine, w, inst.name))
                    inst.sync_info = mybir.SyncInfo(
                        on_wait=[keep], on_update=list(si.on_update)
                    )
                    changed = True
                out.append(inst)
            if changed:
                try:
                    bb.instructions = out
                except Exception:
                    del bb.instructions[:]
                    bb.instructions.extend(out)
    return nc


# ---------------------------------------------------------------- build
def build():
    nc = bass.Bass(trn_type="TRN2")

    xT_d = nc.dram_tensor("xT", [128, KD, TPC], BF16, kind="ExternalInput")
    wg_d = nc.dram_tensor("wg", [128, KD, L], BF16, kind="ExternalInput")
    emw_d = nc.dram_tensor("emw", [KE, 128, KD * 128], BF16, kind="ExternalInput")
    ebs_d = nc.dram_tensor("ebs", [128, KE], F32, kind="ExternalInput")
    lw_d = [
        nc.dram_tensor(f"lw{i}", [KE, 128, KE * 128], BF16, kind="ExternalInput")
        for i in range(2)
    ]
    lb_d = [
        nc.dram_tensor(f"lb{i}", [128, KE], F32, kind="ExternalInput")
        for i in range(2)
    ]
    sel_d = nc.dram_tensor("sel", [L, 128, 128], BF16, kind="ExternalInput")
    ow_d = nc.dram_tensor("ow", [VT, 128, KE * 128], BF16, kind="ExternalInput")
    ob_d = nc.dram_tensor("ob", [128, VT], F32, kind="ExternalInput")
    out_d = nc.dram_tensor("out", [VT, 128, TPC], F32, kind="ExternalOutput")

    with TileContext(nc) as tc:
        with (
            tc.tile_pool(name="xpool", bufs=1) as xpool,
            tc.tile_pool(name="hpool", bufs=1) as hpool,
            tc.tile_pool(name="cpool", bufs=1) as cpool,
            tc.tile_pool(name="wstream", bufs=4) as wstream,
            tc.tile_pool(name="res", bufs=4) as resp,
            tc.tile_pool(name="ps", bufs=4, space="PSUM") as ps,
            tc.tile_pool(name="psg", bufs=2, space="PSUM") as psg,
        ):
            # ---- load x (feature-major) and constants
            xT = [xpool.tile([128, TPC], BF16, tag=f"xT{k}", name=f"xT{k}") for k in range(KD)]
            for k in range(KD):
                nc.sync.dma_start(xT[k][:], xT_d[:, k, :])
            wg_sb = cpool.tile([128, KD, L], BF16)
            nc.sync.dma_start(wg_sb[:], wg_d[:, :, :])
            ebs_sb = cpool.tile([128, KE], F32)
            nc.sync.dma_start(ebs_sb[:], ebs_d[:, :])
            lb_sb = []
            for i in range(2):
                t = cpool.tile([128, KE], F32, tag=f"lb{i}")
                nc.sync.dma_start(t[:], lb_d[i][:, :])
                lb_sb.append(t)
            ob_sb = cpool.tile([128, VT], F32)
            nc.sync.dma_start(ob_sb[:], ob_d[:, :])

            # ---- gates: psum_g[3, TPC] = sum_k wg[k].T @ xT[k]
            psum_g = psg.tile([L, TPC], F32)
            for k in range(KD):
                nc.tensor.matmul(
                    psum_g[:], wg_sb[:, k, :], xT[k][:],
                    start=(k == 0), stop=(k == KD - 1),
                )
            g_sb = cpool.tile([128, TPC], BF16)
            nc.vector.memset(g_sb[:], 0.0)
            nc.scalar.activation(g_sb[0:L, :], psum_g[:], AF.Sigmoid)

            # ---- broadcast g rows across partitions via selector matmuls
            G = []
            for l in range(L):
                sel = cpool.tile([128, 128], BF16, tag=f"sel{l}", name=f"sel{l}")
                nc.sync.dma_start(sel[:], sel_d[l, :, :])
                psum_G = psg.tile([128, TPC], F32, tag="psG")
                nc.tensor.matmul(psum_G[:], sel[:], g_sb[:], start=True, stop=True)
                Gt = cpool.tile([128, TPC], BF16, tag=f"G{l}")
                nc.vector.tensor_copy(Gt[:], psum_G[:])
                G.append(Gt)

            # ---- x' = x * g (per 1024-block)
            xp = [xpool.tile([128, TPC], BF16, tag=f"xp{k}", name=f"xp{k}") for k in range(KD)]
            for k in range(KD):
                nc.vector.tensor_mul(xp[k][:], xT[k][:], G[k // (D_IN // 128)][:])

            # ---- emb GEMM: h[m] = relu(sum_k emw[k,m].T @ xp[k] + ebs[m])
            h = [hpool.tile([128, TPC], BF16, tag=f"h{m}", name=f"h{m}") for m in range(KE)]
            for m in range(KE):
                wt = wstream.tile([128, KD * 128], BF16, tag="wstream")
                nc.sync.dma_start(wt[:], emw_d[m, :, :])
                psum = ps.tile([128, TPC], F32)
                for k in range(KD):
                    nc.tensor.matmul(
                        psum[:], wt[:, k * 128 : (k + 1) * 128], xp[k][:],
                        start=(k == 0), stop=(k == KD - 1),
                    )
                nc.scalar.activation(
                    h[m][:], psum[:], AF.Relu, bias=ebs_sb[:, m : m + 1]
                )

            # ---- two tanh linear layers
            cur = h
            for i in range(2):
                nxt = [
                    hpool.tile([128, TPC], BF16, tag=f"h{i+1}_{m}", name=f"h{i+1}_{m}")
                    for m in range(KE)
                ]
                for m in range(KE):
                    wt = wstream.tile([128, KD * 128], BF16, tag="wstream")
                    nc.sync.dma_start(wt[:, : KE * 128], lw_d[i][m, :, :])
                    psum = ps.tile([128, TPC], F32)
                    for k in range(KE):
                        nc.tensor.matmul(
                            psum[:], wt[:, k * 128 : (k + 1) * 128], cur[k][:],
                            start=(k == 0), stop=(k == KE - 1),
                        )
                    nc.scalar.activation(
                        nxt[m][:], psum[:], AF.Tanh, bias=lb_sb[i][:, m : m + 1]
                    )
                cur = nxt

            # ---- logits GEMM, vocab-major, streamed out_w
            for vt in range(VT):
                wt = wstream.tile([128, KD * 128], BF16, tag="wstream")
                nc.sync.dma_start(wt[:, : KE * 128], ow_d[vt, :, :])
                psum = ps.tile([128, TPC], F32)
                for k in range(KE):
                    nc.tensor.matmul(
                        psum[:], wt[:, k * 128 : (k + 1) * 128], cur[k][:],
                        start=(k == 0), stop=(k == KE - 1),
                    )
                res = resp.tile([128, TPC], F32, tag="res")
                nc.scalar.activation(
                    res[:], psum[:], AF.Identity, bias=ob_sb[:, vt : vt + 1]
                )
                nc.sync.dma_start(out_d[vt, :, :], res[:])

    legalize_waits(nc)
    return nc


_NC_CACHE = []
LAST_EXEC_NS = None
LAST_SPMD_WALL_NS = None


def kernel(x, w, emb_w, emb_b, lin_w, lin_b, out_w, out_b):
    x = np.asarray(x, dtype=np.float32)
    w = np.asarray(w, dtype=np.float32)
    emb_w = np.asarray(emb_w, dtype=np.float32)
    emb_b = np.asarray(emb_b, dtype=np.float32)
    lin_w = np.asarray(lin_w, dtype=np.float32)
    lin_b = np.asarray(lin_b, dtype=np.float32)
    out_w = np.asarray(out_w, dtype=np.float32)
    out_b = np.asarray(out_b, dtype=np.float32)

    bf = ml_dtypes.bfloat16

    # ---- host-side weight prep (shared across cores)
    # gates lhsT: [128, KD, L], wg[p,k,l] = w[l, k*128+p]
    wg = np.ascontiguousarray(
        w.T.reshape(KD, 128, L).transpose(1, 0, 2)
    ).astype(bf)
    # emb weights: emw[m, p, k*128+j] = W[k*128+p, m*128+j], W = [3072, 2048]
    We = emb_w.reshape(D, EMB)
    emw = np.ascontiguousarray(
        We.reshape(KD, 128, KE, 128).transpose(2, 1, 0, 3).reshape(KE, 128, KD * 128)
    ).astype(bf)
    ebs = emb_b.sum(axis=0).reshape(KE, 128).T.astype(np.float32)  # [128, KE]
    ebs = np.ascontiguousarray(ebs)
    lw = []
    lb = []
    for i in range(2):
        Wl = lin_w[i]
        lw.append(
            np.ascontiguousarray(
                Wl.reshape(KE, 128, KE, 128)
                .transpose(2, 1, 0, 3)
                .reshape(KE, 128, KE * 128)
            ).astype(bf)
        )
        lb.append(
            np.ascontiguousarray(lin_b[i].reshape(KE, 128).T.astype(np.float32))
        )
    ow = np.ascontiguousarray(
        out_w.reshape(KE, 128, VT, 128).transpose(2, 1, 0, 3).reshape(VT, 128, KE * 128)
    ).astype(bf)
    ob = np.ascontiguousarray(out_b.reshape(VT, 128).T.astype(np.float32))
    selc = np.zeros((L, 128, 128), dtype=bf)
    for l in range(L):
        selc[l, l, :] = 1

    # ---- per-core token slices, feature-major bf16
    xf = x.reshape(NTOK, D)
    in_maps = []
    for c in range(NCORES):
        xc = xf[c * TPC : (c + 1) * TPC]  # [TPC, D]
        xTc = np.ascontiguousarray(
            xc.T.reshape(KD, 128, TPC).transpose(1, 0, 2)
        ).astype(bf)
        in_maps.append(
            {
                "xT": xTc,
                "wg": wg,
                "emw": emw,
                "ebs": ebs,
                "lw0": lw[0],
                "lw1": lw[1],
                "lb0": lb[0],
                "lb1": lb[1],
                "sel": selc,
                "ow": ow,
                "ob": ob,
            }
        )

    if not _NC_CACHE:
        _NC_CACHE.append(build())
    nc = _NC_CACHE[0]

    import os, time as _time
    trace = bool(os.environ.get("KERNEL_TRACE"))
    t0 = _time.perf_counter()
    try:
        res = run_bass_kernel_spmd(
            nc, in_maps, core_ids=list(range(NCORES)), trace=trace
        )
    except Exception:
        if not trace:
            raise
        res = run_bass_kernel_spmd(nc, in_maps, core_ids=list(range(NCORES)))
    t1 = _time.perf_counter()
    global LAST_EXEC_NS, LAST_SPMD_WALL_NS
    LAST_EXEC_NS = res.exec_time_ns
    LAST_SPMD_WALL_NS = int((t1 - t0) * 1e9)

    # ---- reassemble: out[c] is [VT, 128, TPC] vocab-major
    logits = np.empty((NTOK, OUT), dtype=np.float32)
    for c in range(NCORES):
        oc = res.results[c]["out"]  # [VT, 128, TPC]
        logits[c * TPC : (c + 1) * TPC] = (
            oc.reshape(OUT, TPC).T
        )
    return logits.reshape(B, T, OUT)


if __name__ == "__main__":
    rng = np.random.default_rng(0)
    ins = {
        "x": rng.standard_normal((B, T, D)).astype(np.float32),
        "w": (rng.standard_normal((L, D)) * 0.02).astype(np.float32),
        "emb_w": (rng.standard_normal((L, D_IN, EMB)) * 0.02).astype(np.float32),
        "emb_b": (rng.standard_normal((L, EMB)) * 0.02).astype(np.float32),
        "lin_w": (rng.standard_normal((2, EMB, EMB)) * 0.02).astype(np.float32),
        "lin_b": (rng.standard_normal((2, EMB)) * 0.02).astype(np.float32),
        "out_w": (rng.standard_normal((EMB, OUT)) * 0.02).astype(np.float32),
        "out_b": (rng.standard_normal((OUT,)) * 0.02).astype(np.float32),
    }
    out = kernel(**ins)
    print("kernel output", out.shape, out.dtype)

